# revision 10
# baseline (speedup 1.0000x reference)
"""Triangular matmul C = triu(triu(A) @ triu(B)) on 8 TRN2 NeuronCores.

N=4096 fp32, viewed as a 32x32 grid of 128x128 blocks; the MAC work is the
block-tetrahedron {I <= K <= J} (5984 blocks of 128^3).

Sharding is 2D over the output: column "phases" (512-wide J-groups) are
split into two classes CLS = {0,3,5,6} / {1,2,4,7} carrying exactly half
the MACs each; within a class, rows go to 4 cores per ROWS_TBL (sets found
by local search balancing per-core max(compute, DMA) — ~748 MAC-blocks and
~13 MB of HBM traffic per core).  Core c = (row set c%4, class c//4).

Numerics: operands are rounded to bf16 on the host and each block product
is a single bf16 matmul accumulating in fp32 PSUM (rel err ~2e-3 vs the
fp32 reference; the harness gate is 2e-2).  C is staged to fp16 in SBUF
and upcast on the host.

Schedule per core: phases BIG-FIRST (descending), each phase row-outer
with rows descending, and every B panel loaded in strip-descending ~0.8 MB
chunks: row i's sweep (q = i..4p+3 ascending, start at q == i, stop at
q == 4p+3) needs exactly the strips the DMA stream has already delivered,
so compute starts ~5 us in and the remaining ~12 MB of loads hide behind
the big phases' matmuls (ascending order instead front-loads DMA demand
and idles the PE long enough to re-throttle its clock).  A is host-packed
in the same consumption order (rows descending, K ascending within a
row).  Each row's eviction (DVE/ACT fp32->fp16 copy, alternating) overlaps
the remaining rows; stores are one SWDGE DMA per phase, except the last
(tiny) phase which stores per-row on the HWDGE rings — by then all loads
are long queued, so stores can never block one.  A dummy matmul burst at
t=0 warms the PE clock gate, and a Switch computed-goto (with an early
prefetch hint) dispatches the 8 per-core programs without the ~40 us
serial If-chain walk.

The kernel takes FULL (unsharded) inputs and returns the FULL output.
"""

import numpy as np

N = 4096
BLK = 128
NB = 32
PW = 512  # phase width in cols (4 blocks) = one fp32 PSUM bank
N_CORES = 8
MODE = "bf16x1-2d-v4-bigfirst"

CLS = [[0, 3, 5, 6], [1, 2, 4, 7]]
# Row sets per class (4 cores each), from the assignment optimizer.
# (Rows 28-31 in class 0 have no class-0 output and emit nothing there.)
ROWS_TBL = [
    [[3, 7, 9, 14, 17, 20, 22, 24], [0, 6, 8, 10, 25, 26, 28, 31],
     [4, 11, 12, 13, 15, 16, 18, 23], [1, 2, 5, 19, 21, 27, 29, 30]],
    [[0, 1, 12, 24, 25, 26, 29, 31], [2, 3, 8, 13, 21, 22, 27, 30],
     [5, 7, 11, 14, 15, 16, 18, 20], [4, 6, 9, 10, 17, 19, 23, 28]],
]

A_CHUNK = 36  # A-load DMA granularity in slots (~1.1 MB)
B_CHUNK = 3072  # B-load chunk threshold in cols (~0.8 MB)
N_WARM = 12  # dummy warmup matmuls (beat the HAM clock gate)


def _core_rs(c):
    return c % 4, c // 4


def _rows_of(c):
    r, s = _core_rs(c)
    return ROWS_TBL[s][r]


def _phases(c):
    """[(p, active_rows_desc)] in processing order: phases descending."""
    _, s = _core_rs(c)
    out = []
    for p in sorted(CLS[s], reverse=True):
        act = sorted((i for i in _rows_of(c) if i <= 4 * p + 3), reverse=True)
        if act:
            out.append((p, act))
    return out


def _strips(p, m):
    """K-strips (q, col0, width_cols) of phase p starting at row-block m."""
    out = []
    for q in range(m, 4 * p + 4):
        c0 = max(4 * p, q) * BLK
        out.append((q, c0, (4 * p + 4) * BLK - c0))
    return out


def _b_layout(c):
    """{(p, q): within-bpack col offset}, total width, per-phase spans."""
    off, w = {}, 0
    spans = {}
    for p, act in _phases(c):
        p0 = w
        for q, _, wid in _strips(p, act[-1]):
            off[(p, q)] = w
            w += wid
        spans[p] = (p0, w)
    return off, w, spans


def _a_layout(c):
    """Packed-A slots in consumption order: rows descending, K ascending
    within a row (the first, biggest phase sweeps every slot this way)."""
    phs = _phases(c)
    kmax = 4 * phs[0][0] + 3
    rows = sorted(set(i for _, act in phs for i in act), reverse=True)
    slots = {}
    for i in rows:
        for q in range(i, kmax + 1):
            slots[(q, i)] = len(slots)
    return slots


def _c_layout(c):
    """Packed-C 512-col slots: {(p, i): slot}, contiguous per phase."""
    slots = {}
    for p, act in _phases(c):
        for i in act:
            slots[(p, i)] = len(slots)
    return slots


NA_MAX = max(len(_a_layout(c)) for c in range(N_CORES))
WB_MAX = max(_b_layout(c)[1] for c in range(N_CORES))
NC_MAX = max(len(_c_layout(c)) for c in range(N_CORES))


def _emit_core(nc, tc, pools, dram_io, core):
    import concourse.mybir as mybir

    f32 = mybir.dt.float32
    bf16 = mybir.dt.bfloat16
    fp16 = mybir.dt.float16
    apool, bpool, cpool, psum_pool = pools
    apack, bpack, cpack = dram_io["apack"], dram_io["bpack"], dram_io["cpack"]
    aslot = _a_layout(core)
    cslot = _c_layout(core)
    boff, wb, bspans = _b_layout(core)
    phs = _phases(core)
    na = len(aslot)

    # --- PE warmup while the first loads are in flight.
    warm = bpool.tile([BLK, PW], bf16, name="warm", tag="warm")
    nc.gpsimd.memset(warm[:], 0.0)
    wps = psum_pool.tile([BLK, PW], f32, name="warmps", tag="ps7")
    for i in range(N_WARM):
        nc.tensor.matmul(
            wps[:], warm[:, :BLK], warm[:], start=(i == 0), stop=(i == N_WARM - 1)
        )

    # --- loads, in consumption order: B (strip-descending chunks per
    # phase) on the SP ring; A (rows-descending slots) on the ACT ring.
    a_t = apool.tile([BLK, na, BLK], bf16, name="a", tag="a")
    prev_a = 0
    while prev_a < na:
        step = min(A_CHUNK, na - prev_a)
        nc.scalar.dma_start(
            a_t[:, prev_a : prev_a + step, :], apack[:, prev_a : prev_a + step, :]
        )
        prev_a += step

    b_t = {}
    for pi, (p, act) in enumerate(phs):
        p0, p1 = bspans[p]
        b_t[p] = bpool.tile([BLK, p1 - p0], bf16, name=f"b_{p}", tag=f"bp{pi}")
        acc_lo, acc_w = None, 0
        for q, _, wid in reversed(_strips(p, act[-1])):
            acc_lo = boff[(p, q)]
            acc_w += wid
            if acc_w >= B_CHUNK:
                nc.sync.dma_start(
                    b_t[p][:, acc_lo - p0 : acc_lo - p0 + acc_w],
                    bpack[:, acc_lo : acc_lo + acc_w],
                )
                acc_lo, acc_w = None, 0
        if acc_w:
            nc.sync.dma_start(
                b_t[p][:, acc_lo - p0 : acc_lo - p0 + acc_w],
                bpack[:, acc_lo : acc_lo + acc_w],
            )

    # --- compute: phases big-first, rows descending, per-row eviction.
    bank = 0
    copy_eng = [nc.vector.tensor_copy, nc.scalar.copy]
    store_eng = [nc.sync, nc.scalar]
    ncopy = 0
    for pi, (p, act) in enumerate(phs):
        p0, _ = bspans[p]
        last_strip = 4 * p + 3
        last_phase = pi == len(phs) - 1
        cst = None
        if not last_phase:
            cst = cpool.tile(
                [BLK, len(act) * PW], fp16, name=f"cst_{p}", tag=f"cst{pi % 2}"
            )
        s0 = cslot[(p, act[0])]
        for ji, i in enumerate(act):
            pst = psum_pool.tile(
                [BLK, PW], f32, name=f"ps_{p}_{i}", tag=f"ps{bank % 8}"
            )
            bank += 1
            for q, c0, wid in _strips(p, i):
                rel = c0 - 4 * p * BLK
                nc.tensor.matmul(
                    pst[:, rel : rel + wid],
                    a_t[:, aslot[(q, i)], :],
                    b_t[p][:, boff[(p, q)] - p0 : boff[(p, q)] - p0 + wid],
                    start=(q == i),
                    stop=(q == last_strip),
                )
            mr = max(0, i - 4 * p) * BLK
            if last_phase:
                ct = cpool.tile([BLK, PW], fp16, name=f"ct_{i}", tag=f"ct{ji % 2}")
                copy_eng[ncopy % 2](ct[:, mr:PW], pst[:, mr:PW])
                store_eng[ji % 2].dma_start(
                    cpack[:, cslot[(p, i)] * PW + mr : (cslot[(p, i)] + 1) * PW],
                    ct[:, mr:PW],
                )
            else:
                j = cslot[(p, i)] - s0
                copy_eng[ncopy % 2](
                    cst[:, j * PW + mr : (j + 1) * PW], pst[:, mr:PW]
                )
            ncopy += 1
        if not last_phase:
            nc.gpsimd.dma_start(
                cpack[:, s0 * PW : (s0 + len(act)) * PW], cst[:]
            )


def _build():
    import concourse.mybir as mybir
    import concourse.tile as tile
    from concourse import bacc

    nc = bacc.Bacc(None, target_bir_lowering=False, debug=False)
    bf16 = mybir.dt.bfloat16
    fp16 = mybir.dt.float16
    with tile.TileContext(nc) as tc:
        with (
            tc.tile_pool(name="dram", bufs=1, space="DRAM") as dram,
            tc.tile_pool(name="apool", bufs=1) as apool,
            tc.tile_pool(name="bpool", bufs=1) as bpool,
            tc.tile_pool(name="cpool", bufs=1) as cpool,
            tc.tile_pool(name="psum", bufs=1, space="PSUM") as psum_pool,
        ):
            dram_io = {
                "apack": dram.tile(
                    [BLK, NA_MAX, BLK], bf16, kind="ExternalInput",
                    name="apack", uniquify=False,
                ),
                "bpack": dram.tile(
                    [BLK, WB_MAX], bf16, kind="ExternalInput",
                    name="bpack", uniquify=False,
                ),
                "cpack": dram.tile(
                    [BLK, NC_MAX * PW], fp16, kind="ExternalOutput",
                    name="cpack", uniquify=False,
                ),
            }
            pid = nc.partition_id()
            pools = (apool, bpool, cpool, psum_pool)
            tc.switch_hint({e: pid for e in mybir.ALL_ENGINES}, N_CORES, label="core")
            for c in tc.Switch(pid, N_CORES, hint="core"):
                _emit_core(nc, tc, pools, dram_io, c)
    nc.compile()
    return nc


_cached_nc = None

# Optional profiling knobs (used by test.py; harness leaves them off).
TRACE = False
TRACE_KW = {}
LAST_RESULTS = None


def _get_nc():
    global _cached_nc
    if _cached_nc is None:
        _cached_nc = _build()
    return _cached_nc


def _host_pack(A, B):
    import ml_dtypes

    bf16 = ml_dtypes.bfloat16
    AT = np.ascontiguousarray(A.T).astype(bf16)
    Bb = B.astype(bf16)
    apacks, bpacks = [], []
    for c in range(N_CORES):
        ap = np.zeros((BLK, NA_MAX, BLK), dtype=bf16)
        for (q, i), idx in _a_layout(c).items():
            ap[:, idx, :] = AT[q * BLK : (q + 1) * BLK, i * BLK : (i + 1) * BLK]
        bp = np.zeros((BLK, WB_MAX), dtype=bf16)
        boff, _, _ = _b_layout(c)
        for p, act in _phases(c):
            for q, c0, wid in _strips(p, act[-1]):
                w0 = boff[(p, q)]
                bp[:, w0 : w0 + wid] = Bb[q * BLK : (q + 1) * BLK, c0 : c0 + wid]
        apacks.append(ap)
        bpacks.append(bp)
    return apacks, bpacks


def kernel(A, B):
    from concourse.bass_utils import run_bass_kernel_spmd

    A = np.asarray(A, dtype=np.float32)
    B = np.asarray(B, dtype=np.float32)
    nc = _get_nc()
    apacks, bpacks = _host_pack(A, B)
    in_maps = [{"apack": apacks[c], "bpack": bpacks[c]} for c in range(N_CORES)]
    res = run_bass_kernel_spmd(
        nc, in_maps, core_ids=list(range(N_CORES)), trace=TRACE, **TRACE_KW
    )
    global LAST_RESULTS
    LAST_RESULTS = res

    C = np.zeros((N, N), dtype=np.float32)
    for c in range(N_CORES):
        cp = res.results[c]["cpack"]
        for (p, i), j in _c_layout(c).items():
            mr = max(0, i - 4 * p) * BLK
            C[i * BLK : (i + 1) * BLK, p * PW + mr : (p + 1) * PW] = cp[
                :, j * PW + mr : (j + 1) * PW
            ].astype(np.float32)
    return np.triu(C)


# revision 13
# speedup vs baseline: 1.0107x; 1.0107x over previous
"""Triangular matmul C = triu(triu(A) @ triu(B)) on 8 TRN2 NeuronCores.

N=4096 fp32, viewed as a 32x32 grid of 128x128 blocks; the MAC work is the
block-tetrahedron {I <= K <= J} (5984 blocks of 128^3).

Sharding is 2D over the output: column "phases" (512-wide J-groups) are
split into two classes CLS = {0,3,5,6} / {1,2,4,7} carrying exactly half
the MACs each; within a class, rows go to 4 cores per ROWS_TBL (sets found
by local search balancing per-core max(compute, DMA) — ~748 MAC-blocks and
~13 MB of HBM traffic per core).  Core c = (row set c%4, class c//4).

Numerics: operands are rounded to bf16 on the host and each block product
is a single bf16 matmul accumulating in fp32 PSUM (rel err ~2e-3 vs the
fp32 reference; the harness gate is 2e-2).  C is staged to fp16 in SBUF
and upcast on the host.

Schedule per core: phases BIG-FIRST (descending), each phase row-outer
with rows descending, and every B panel loaded in strip-descending ~0.8 MB
chunks: row i's sweep (q = i..4p+3 ascending, start at q == i, stop at
q == 4p+3) needs exactly the strips the DMA stream has already delivered,
so compute starts ~5 us in and the remaining ~12 MB of loads hide behind
the big phases' matmuls (ascending order instead front-loads DMA demand
and idles the PE long enough to re-throttle its clock).  A is host-packed
in the same consumption order (rows descending, K ascending within a
row).  Each row's eviction (DVE/ACT fp32->fp16 copy, alternating) overlaps
the remaining rows; stores are one SWDGE DMA per phase, except the last
(tiny) phase which stores per-row on the HWDGE rings — by then all loads
are long queued, so stores can never block one.  A dummy matmul burst at
t=0 warms the PE clock gate, and a Switch computed-goto (with an early
prefetch hint) dispatches the 8 per-core programs without the ~40 us
serial If-chain walk.

The kernel takes FULL (unsharded) inputs and returns the FULL output.
"""

import numpy as np

N = 4096
BLK = 128
NB = 32
PW = 512  # phase width in cols (4 blocks) = one fp32 PSUM bank
N_CORES = 8
MODE = "bf16x1-2d-v4-bigfirst"

CLS = [[0, 3, 5, 6], [1, 2, 4, 7]]
# Row sets per class (4 cores each), from the assignment optimizer.
# (Rows 28-31 in class 0 have no class-0 output and emit nothing there.)
ROWS_TBL = [
    [[3, 7, 9, 14, 17, 20, 22, 24], [0, 6, 8, 10, 25, 26, 28, 31],
     [4, 11, 12, 13, 15, 16, 18, 23], [1, 2, 5, 19, 21, 27, 29, 30]],
    [[0, 1, 12, 24, 25, 26, 29, 31], [2, 3, 8, 13, 21, 22, 27, 30],
     [5, 7, 11, 14, 15, 16, 18, 20], [4, 6, 9, 10, 17, 19, 23, 28]],
]

A_CHUNK = 36  # A-load DMA granularity in slots (~1.1 MB)
B_CHUNK = 3072  # B-load chunk threshold in cols (~0.8 MB)
N_WARM = 12  # dummy warmup matmuls (beat the HAM clock gate)


def _core_rs(c):
    return c % 4, c // 4


def _rows_of(c):
    r, s = _core_rs(c)
    return ROWS_TBL[s][r]


def _phases(c):
    """[(p, active_rows_desc)] in processing order: phases descending."""
    _, s = _core_rs(c)
    out = []
    for p in sorted(CLS[s], reverse=True):
        act = sorted((i for i in _rows_of(c) if i <= 4 * p + 3), reverse=True)
        if act:
            out.append((p, act))
    return out


def _strips(p, m):
    """K-strips (q, col0, width_cols) of phase p starting at row-block m."""
    out = []
    for q in range(m, 4 * p + 4):
        c0 = max(4 * p, q) * BLK
        out.append((q, c0, (4 * p + 4) * BLK - c0))
    return out


def _b_layout(c):
    """{(p, q): within-bpack col offset}, total width, per-phase spans."""
    off, w = {}, 0
    spans = {}
    for p, act in _phases(c):
        p0 = w
        for q, _, wid in _strips(p, act[-1]):
            off[(p, q)] = w
            w += wid
        spans[p] = (p0, w)
    return off, w, spans


def _a_layout(c):
    """Packed-A slots in consumption order: rows descending, K ascending
    within a row (the first, biggest phase sweeps every slot this way)."""
    phs = _phases(c)
    kmax = 4 * phs[0][0] + 3
    rows = sorted(set(i for _, act in phs for i in act), reverse=True)
    slots = {}
    for i in rows:
        for q in range(i, kmax + 1):
            slots[(q, i)] = len(slots)
    return slots


def _c_layout(c):
    """Packed-C 512-col slots: {(p, i): slot}, contiguous per phase."""
    slots = {}
    for p, act in _phases(c):
        for i in act:
            slots[(p, i)] = len(slots)
    return slots


NA_MAX = max(len(_a_layout(c)) for c in range(N_CORES))
WB_MAX = max(_b_layout(c)[1] for c in range(N_CORES))
NC_MAX = max(len(_c_layout(c)) for c in range(N_CORES))


def _emit_core(nc, tc, pools, dram_io, core):
    import concourse.mybir as mybir

    f32 = mybir.dt.float32
    bf16 = mybir.dt.bfloat16
    fp16 = mybir.dt.float16
    apool, bpool, cpool, psum_pool = pools
    apack, bpack, cpack = dram_io["apack"], dram_io["bpack"], dram_io["cpack"]
    aslot = _a_layout(core)
    cslot = _c_layout(core)
    boff, wb, bspans = _b_layout(core)
    phs = _phases(core)
    na = len(aslot)

    # --- PE warmup while the first loads are in flight.
    warm = bpool.tile([BLK, PW], bf16, name="warm", tag="warm")
    nc.gpsimd.memset(warm[:], 0.0)
    wps = psum_pool.tile([BLK, PW], f32, name="warmps", tag="ps7")
    for i in range(N_WARM):
        nc.tensor.matmul(
            wps[:], warm[:, :BLK], warm[:], start=(i == 0), stop=(i == N_WARM - 1)
        )

    # --- loads are emitted JUST-IN-TIME, interleaved with the compute
    # stream a few rows ahead of consumption.  Tile attaches a reader's
    # dependency to every already-emitted writer of the tile, so chunks
    # issued upfront would make the first matmul wait for the LAST chunk;
    # emitting each chunk right before the rows that need it keeps the
    # dependency minimal while the ring still runs ahead of the PE.
    a_t = apool.tile([BLK, na, BLK], bf16, name="a", tag="a")
    b_t = {}
    for pi, (p, act) in enumerate(phs):
        p0, p1 = bspans[p]
        b_t[p] = bpool.tile([BLK, p1 - p0], bf16, name=f"b_{p}", tag=f"bp{pi}")

    # chunk plans, in consumption order
    a_chunks = []  # (lo, hi) slot ranges
    lo = 0
    while lo < na:
        a_chunks.append((lo, min(lo + A_CHUNK, na)))
        lo += A_CHUNK
    b_chunks = {}  # p -> [(col_lo, width)] strip-descending
    for p, act in phs:
        ch = []
        acc_lo, acc_w = None, 0
        for q, _, wid in reversed(_strips(p, act[-1])):
            acc_lo = boff[(p, q)]
            acc_w += wid
            if acc_w >= B_CHUNK:
                ch.append((acc_lo, acc_w))
                acc_lo, acc_w = None, 0
        if acc_w:
            ch.append((acc_lo, acc_w))
        b_chunks[p] = ch

    a_issued = 0  # chunks emitted so far
    b_issued = {p: 0 for p, _ in phs}

    def need_a(slot_hi):
        nonlocal a_issued
        while a_issued < len(a_chunks) and (
            a_issued == 0 or a_chunks[a_issued - 1][1] <= slot_hi
        ):
            clo, chi = a_chunks[a_issued]
            nc.scalar.dma_start(
                a_t[:, clo:chi, :], apack[:, clo:chi, :]
            )
            a_issued += 1

    def need_b(p, col_lo):
        """Issue phase-p chunks (desc) until col_lo is covered."""
        ch = b_chunks[p]
        p0 = bspans[p][0]
        while b_issued[p] < len(ch):
            clo, cw = ch[b_issued[p]]
            if b_issued[p] > 0 and ch[b_issued[p] - 1][0] <= col_lo:
                break
            nc.sync.dma_start(
                b_t[p][:, clo - p0 : clo - p0 + cw], bpack[:, clo : clo + cw]
            )
            b_issued[p] += 1

    # rows in processing order with a lookahead-driven chunk issue
    seq = [(pi, p, act, i) for pi, (p, act) in enumerate(phs) for i in act]
    LA = 3

    def prefetch_for(j):
        if j >= len(seq):
            return
        _pi, p, act, i = seq[j]
        need_b(p, boff[(p, i)])
        amax = max(aslot[(q, i)] for q, _, _ in _strips(p, i))
        need_a(amax + 1)

    for j in range(LA):
        prefetch_for(j)

    # --- compute: phases big-first, rows descending, per-row eviction.
    # Final-phase stores avoid the SP ring: B loads still issue there, and
    # a waiting store would block them (FIFO per sequencer).
    bank = 0
    store_eng = [nc.scalar, nc.gpsimd]
    cst = {}
    for j, (pi, p, act, i) in enumerate(seq):
        prefetch_for(j + LA)
        p0, _ = bspans[p]
        last_strip = 4 * p + 3
        last_phase = pi == len(phs) - 1
        if not last_phase and p not in cst:
            cst[p] = cpool.tile(
                [BLK, len(act) * PW], fp16, name=f"cst_{p}", tag=f"cst{pi % 2}"
            )
        pst = psum_pool.tile([BLK, PW], f32, name=f"ps_{p}_{i}", tag=f"ps{bank % 8}")
        bank += 1
        for q, c0, wid in _strips(p, i):
            rel = c0 - 4 * p * BLK
            nc.tensor.matmul(
                pst[:, rel : rel + wid],
                a_t[:, aslot[(q, i)], :],
                b_t[p][:, boff[(p, q)] - p0 : boff[(p, q)] - p0 + wid],
                start=(q == i),
                stop=(q == last_strip),
            )
        mr = max(0, i - 4 * p) * BLK
        if last_phase:
            ji = act.index(i)
            ct = cpool.tile([BLK, PW], fp16, name=f"ct_{i}", tag=f"ct{ji % 2}")
            nc.vector.tensor_copy(ct[:, mr:PW], pst[:, mr:PW])
            store_eng[ji % 2].dma_start(
                cpack[:, cslot[(p, i)] * PW + mr : (cslot[(p, i)] + 1) * PW],
                ct[:, mr:PW],
            )
        else:
            s0 = cslot[(p, act[0])]
            jrow = cslot[(p, i)] - s0
            nc.vector.tensor_copy(
                cst[p][:, jrow * PW + mr : (jrow + 1) * PW], pst[:, mr:PW]
            )
            if i == act[-1]:
                nc.gpsimd.dma_start(
                    cpack[:, s0 * PW : (s0 + len(act)) * PW], cst[p][:]
                )


def _build():
    import concourse.mybir as mybir
    import concourse.tile as tile
    from concourse import bacc

    nc = bacc.Bacc(None, target_bir_lowering=False, debug=False)
    bf16 = mybir.dt.bfloat16
    fp16 = mybir.dt.float16
    with tile.TileContext(nc) as tc:
        with (
            tc.tile_pool(name="dram", bufs=1, space="DRAM") as dram,
            tc.tile_pool(name="apool", bufs=1) as apool,
            tc.tile_pool(name="bpool", bufs=1) as bpool,
            tc.tile_pool(name="cpool", bufs=1) as cpool,
            tc.tile_pool(name="psum", bufs=1, space="PSUM") as psum_pool,
        ):
            dram_io = {
                "apack": dram.tile(
                    [BLK, NA_MAX, BLK], bf16, kind="ExternalInput",
                    name="apack", uniquify=False,
                ),
                "bpack": dram.tile(
                    [BLK, WB_MAX], bf16, kind="ExternalInput",
                    name="bpack", uniquify=False,
                ),
                "cpack": dram.tile(
                    [BLK, NC_MAX * PW], fp16, kind="ExternalOutput",
                    name="cpack", uniquify=False,
                ),
            }
            pid = nc.partition_id()
            pools = (apool, bpool, cpool, psum_pool)
            tc.switch_hint({e: pid for e in mybir.ALL_ENGINES}, N_CORES, label="core")
            for c in tc.Switch(pid, N_CORES, hint="core"):
                _emit_core(nc, tc, pools, dram_io, c)
    nc.compile()
    return nc


_cached_nc = None

# Optional profiling knobs (used by test.py; harness leaves them off).
TRACE = False
TRACE_KW = {}
LAST_RESULTS = None


def _get_nc():
    global _cached_nc
    if _cached_nc is None:
        _cached_nc = _build()
    return _cached_nc


def _host_pack(A, B):
    import ml_dtypes

    bf16 = ml_dtypes.bfloat16
    AT = np.ascontiguousarray(A.T).astype(bf16)
    Bb = B.astype(bf16)
    apacks, bpacks = [], []
    for c in range(N_CORES):
        ap = np.zeros((BLK, NA_MAX, BLK), dtype=bf16)
        for (q, i), idx in _a_layout(c).items():
            ap[:, idx, :] = AT[q * BLK : (q + 1) * BLK, i * BLK : (i + 1) * BLK]
        bp = np.zeros((BLK, WB_MAX), dtype=bf16)
        boff, _, _ = _b_layout(c)
        for p, act in _phases(c):
            for q, c0, wid in _strips(p, act[-1]):
                w0 = boff[(p, q)]
                bp[:, w0 : w0 + wid] = Bb[q * BLK : (q + 1) * BLK, c0 : c0 + wid]
        apacks.append(ap)
        bpacks.append(bp)
    return apacks, bpacks


def kernel(A, B):
    from concourse.bass_utils import run_bass_kernel_spmd

    A = np.asarray(A, dtype=np.float32)
    B = np.asarray(B, dtype=np.float32)
    nc = _get_nc()
    apacks, bpacks = _host_pack(A, B)
    in_maps = [{"apack": apacks[c], "bpack": bpacks[c]} for c in range(N_CORES)]
    res = run_bass_kernel_spmd(
        nc, in_maps, core_ids=list(range(N_CORES)), trace=TRACE, **TRACE_KW
    )
    global LAST_RESULTS
    LAST_RESULTS = res

    C = np.zeros((N, N), dtype=np.float32)
    for c in range(N_CORES):
        cp = res.results[c]["cpack"]
        for (p, i), j in _c_layout(c).items():
            mr = max(0, i - 4 * p) * BLK
            C[i * BLK : (i + 1) * BLK, p * PW + mr : (p + 1) * PW] = cp[
                :, j * PW + mr : (j + 1) * PW
            ].astype(np.float32)
    return np.triu(C)


# revision 16
# speedup vs baseline: 1.0630x; 1.0517x over previous
"""Triangular matmul C = triu(triu(A) @ triu(B)) on 8 TRN2 NeuronCores.

N=4096 fp32, viewed as a 32x32 grid of 128x128 blocks; the MAC work is the
block-tetrahedron {I <= K <= J} (5984 blocks of 128^3).

Sharding is 2D over the output: column "phases" (512-wide J-groups) are
split into two classes CLS = {0,3,5,6} / {1,2,4,7} carrying exactly half
the MACs each; within a class, rows go to 4 cores per ROWS_TBL (sets found
by local search balancing per-core max(compute, DMA) — ~748 MAC-blocks and
~13 MB of HBM traffic per core).  Core c = (row set c%4, class c//4).

Numerics: operands are rounded to bf16 on the host and each block product
is a single bf16 matmul accumulating in fp32 PSUM (rel err ~2e-3 vs the
fp32 reference; the harness gate is 2e-2).  C is staged to fp16 in SBUF
and upcast on the host.

Schedule per core: phases BIG-FIRST (descending), each phase row-outer
with rows descending, and every B panel loaded in strip-descending ~0.8 MB
chunks: row i's sweep (q = i..4p+3 ascending, start at q == i, stop at
q == 4p+3) needs exactly the strips the DMA stream has already delivered,
so compute starts ~5 us in and the remaining ~12 MB of loads hide behind
the big phases' matmuls (ascending order instead front-loads DMA demand
and idles the PE long enough to re-throttle its clock).  A is host-packed
in the same consumption order (rows descending, K ascending within a
row).  Each row's eviction (DVE/ACT fp32->fp16 copy, alternating) overlaps
the remaining rows; stores are one SWDGE DMA per phase, except the last
(tiny) phase which stores per-row on the HWDGE rings — by then all loads
are long queued, so stores can never block one.  A dummy matmul burst at
t=0 warms the PE clock gate, and a Switch computed-goto (with an early
prefetch hint) dispatches the 8 per-core programs without the ~40 us
serial If-chain walk.

The kernel takes FULL (unsharded) inputs and returns the FULL output.
"""

import numpy as np

N = 4096
BLK = 128
NB = 32
PW = 512  # phase width in cols (4 blocks) = one fp32 PSUM bank
N_CORES = 8
MODE = "bf16x1-2d-v4-bigfirst"

CLS = [[0, 3, 5, 6], [1, 2, 4, 7]]
# Row sets per class (4 cores each), from the assignment optimizer.
# (Rows 28-31 in class 0 have no class-0 output and emit nothing there.)
ROWS_TBL = [
    [[3, 7, 9, 14, 17, 20, 22, 24], [0, 6, 8, 10, 25, 26, 28, 31],
     [4, 11, 12, 13, 15, 16, 18, 23], [1, 2, 5, 19, 21, 27, 29, 30]],
    [[0, 1, 12, 24, 25, 26, 29, 31], [2, 3, 8, 13, 21, 22, 27, 30],
     [5, 7, 11, 14, 15, 16, 18, 20], [4, 6, 9, 10, 17, 19, 23, 28]],
]

A_CHUNK = 36  # A-load DMA granularity in slots (~1.1 MB)
B_CHUNK = 3072  # B-load chunk threshold in cols (~0.8 MB)
N_WARM = 16  # dummy warmup matmuls (beat the HAM clock gate)


def _core_rs(c):
    return c % 4, c // 4


def _rows_of(c):
    r, s = _core_rs(c)
    return ROWS_TBL[s][r]


def _phases(c):
    """[(p, active_rows_desc)] in processing order: phases descending."""
    _, s = _core_rs(c)
    out = []
    for p in sorted(CLS[s], reverse=True):
        act = sorted((i for i in _rows_of(c) if i <= 4 * p + 3), reverse=True)
        if act:
            out.append((p, act))
    return out


def _strips(p, m):
    """K-strips (q, col0, width_cols) of phase p starting at row-block m."""
    out = []
    for q in range(m, 4 * p + 4):
        c0 = max(4 * p, q) * BLK
        out.append((q, c0, (4 * p + 4) * BLK - c0))
    return out


def _b_layout(c):
    """{(p, q): within-bpack col offset}, total width, per-phase spans."""
    off, w = {}, 0
    spans = {}
    for p, act in _phases(c):
        p0 = w
        for q, _, wid in _strips(p, act[-1]):
            off[(p, q)] = w
            w += wid
        spans[p] = (p0, w)
    return off, w, spans


def _a_layout(c):
    """Packed-A slots in consumption order: rows descending, K ascending
    within a row (the first, biggest phase sweeps every slot this way)."""
    phs = _phases(c)
    kmax = 4 * phs[0][0] + 3
    rows = sorted(set(i for _, act in phs for i in act), reverse=True)
    slots = {}
    for i in rows:
        for q in range(i, kmax + 1):
            slots[(q, i)] = len(slots)
    return slots


def _c_layout(c):
    """Packed-C 512-col slots: {(p, i): slot}, contiguous per phase."""
    slots = {}
    for p, act in _phases(c):
        for i in act:
            slots[(p, i)] = len(slots)
    return slots


NA_MAX = max(len(_a_layout(c)) for c in range(N_CORES))
WB_MAX = max(_b_layout(c)[1] for c in range(N_CORES))
NC_MAX = max(len(_c_layout(c)) for c in range(N_CORES))


def _emit_core(nc, tc, pools, dram_io, core):
    import concourse.mybir as mybir

    f32 = mybir.dt.float32
    bf16 = mybir.dt.bfloat16
    fp16 = mybir.dt.float16
    apool, bpool, cpool, psum_pool = pools
    apack, bpack, cpack = dram_io["apack"], dram_io["bpack"], dram_io["cpack"]
    aslot = _a_layout(core)
    cslot = _c_layout(core)
    boff, wb, bspans = _b_layout(core)
    phs = _phases(core)
    na = len(aslot)

    # --- PE warmup while the first loads are in flight.
    warm = bpool.tile([BLK, PW], bf16, name="warm", tag="warm")
    nc.gpsimd.memset(warm[:], 0.0)
    wps = psum_pool.tile([BLK, PW], f32, name="warmps", tag="ps7")
    for i in range(N_WARM):
        nc.tensor.matmul(
            wps[:], warm[:, :BLK], warm[:], start=(i == 0), stop=(i == N_WARM - 1)
        )

    # --- loads are emitted JUST-IN-TIME, interleaved with the compute
    # stream a few rows ahead of consumption.  Tile attaches a reader's
    # dependency to every already-emitted writer of the tile, so chunks
    # issued upfront would make the first matmul wait for the LAST chunk;
    # emitting each chunk right before the rows that need it keeps the
    # dependency minimal while the ring still runs ahead of the PE.
    a_t = apool.tile([BLK, na, BLK], bf16, name="a", tag="a")
    b_t = {}
    for pi, (p, act) in enumerate(phs):
        p0, p1 = bspans[p]
        b_t[p] = bpool.tile([BLK, p1 - p0], bf16, name=f"b_{p}", tag=f"bp{pi}")

    # chunk plans, in consumption order (first A chunks small so the first
    # rows' weights land quickly behind the ~5 us DMA arming latency)
    a_chunks = []  # (lo, hi) slot ranges
    lo = 0
    while lo < na:
        step = 12 if len(a_chunks) < 2 else A_CHUNK
        a_chunks.append((lo, min(lo + step, na)))
        lo += step
    b_chunks = {}  # p -> [(col_lo, width)] strip-descending
    for p, act in phs:
        ch = []
        acc_lo, acc_w = None, 0
        for q, _, wid in reversed(_strips(p, act[-1])):
            acc_lo = boff[(p, q)]
            acc_w += wid
            if acc_w >= B_CHUNK:
                ch.append((acc_lo, acc_w))
                acc_lo, acc_w = None, 0
        if acc_w:
            ch.append((acc_lo, acc_w))
        b_chunks[p] = ch

    a_issued = 0  # chunks emitted so far
    b_issued = {p: 0 for p, _ in phs}

    def need_a(slot_hi):
        # Same (SP) ring as B: the SDMA inter-queue arbitration starves a
        # second HWDGE ring when the first is busy, so a separate A ring
        # delivers ~70 GB/s; one ring in consumption order paces exactly.
        nonlocal a_issued
        while a_issued < len(a_chunks) and (
            a_issued == 0 or a_chunks[a_issued - 1][1] <= slot_hi
        ):
            clo, chi = a_chunks[a_issued]
            nc.sync.dma_start(
                a_t[:, clo:chi, :], apack[:, clo:chi, :]
            )
            a_issued += 1

    def need_b(p, col_lo):
        """Issue phase-p chunks (desc) until col_lo is covered."""
        ch = b_chunks[p]
        p0 = bspans[p][0]
        while b_issued[p] < len(ch):
            clo, cw = ch[b_issued[p]]
            if b_issued[p] > 0 and ch[b_issued[p] - 1][0] <= col_lo:
                break
            nc.sync.dma_start(
                b_t[p][:, clo - p0 : clo - p0 + cw], bpack[:, clo : clo + cw]
            )
            b_issued[p] += 1

    # rows in processing order with a lookahead-driven chunk issue
    seq = [(pi, p, act, i) for pi, (p, act) in enumerate(phs) for i in act]
    LA = 3

    def prefetch_for(j):
        if j >= len(seq):
            return
        _pi, p, act, i = seq[j]
        need_b(p, boff[(p, i)])
        amax = max(aslot[(q, i)] for q, _, _ in _strips(p, i))
        need_a(amax + 1)

    for j in range(LA):
        prefetch_for(j)

    # --- compute: phases big-first, rows descending, per-row eviction.
    # Final-phase stores avoid the SP ring: B loads still issue there, and
    # a waiting store would block them (FIFO per sequencer).
    bank = 0
    store_eng = [nc.scalar, nc.gpsimd]
    cst = {}
    for j, (pi, p, act, i) in enumerate(seq):
        prefetch_for(j + LA)
        p0, _ = bspans[p]
        last_strip = 4 * p + 3
        last_phase = pi == len(phs) - 1
        if not last_phase and p not in cst:
            cst[p] = cpool.tile(
                [BLK, len(act) * PW], fp16, name=f"cst_{p}", tag=f"cst{pi % 2}"
            )
        pst = psum_pool.tile([BLK, PW], f32, name=f"ps_{p}_{i}", tag=f"ps{bank % 8}")
        bank += 1
        for q, c0, wid in _strips(p, i):
            rel = c0 - 4 * p * BLK
            nc.tensor.matmul(
                pst[:, rel : rel + wid],
                a_t[:, aslot[(q, i)], :],
                b_t[p][:, boff[(p, q)] - p0 : boff[(p, q)] - p0 + wid],
                start=(q == i),
                stop=(q == last_strip),
            )
        mr = max(0, i - 4 * p) * BLK
        if last_phase:
            ji = act.index(i)
            ct = cpool.tile([BLK, PW], fp16, name=f"ct_{i}", tag=f"ct{ji % 2}")
            nc.vector.tensor_copy(ct[:, mr:PW], pst[:, mr:PW])
            store_eng[ji % 2].dma_start(
                cpack[:, cslot[(p, i)] * PW + mr : (cslot[(p, i)] + 1) * PW],
                ct[:, mr:PW],
            )
        else:
            s0 = cslot[(p, act[0])]
            jrow = cslot[(p, i)] - s0
            nc.vector.tensor_copy(
                cst[p][:, jrow * PW + mr : (jrow + 1) * PW], pst[:, mr:PW]
            )
            if i == act[-1]:
                nc.gpsimd.dma_start(
                    cpack[:, s0 * PW : (s0 + len(act)) * PW], cst[p][:]
                )


def _build():
    import concourse.mybir as mybir
    import concourse.tile as tile
    from concourse import bacc

    nc = bacc.Bacc(None, target_bir_lowering=False, debug=False)
    bf16 = mybir.dt.bfloat16
    fp16 = mybir.dt.float16
    with tile.TileContext(nc) as tc:
        with (
            tc.tile_pool(name="dram", bufs=1, space="DRAM") as dram,
            tc.tile_pool(name="apool", bufs=1) as apool,
            tc.tile_pool(name="bpool", bufs=1) as bpool,
            tc.tile_pool(name="cpool", bufs=1) as cpool,
            tc.tile_pool(name="psum", bufs=1, space="PSUM") as psum_pool,
        ):
            dram_io = {
                "apack": dram.tile(
                    [BLK, NA_MAX, BLK], bf16, kind="ExternalInput",
                    name="apack", uniquify=False,
                ),
                "bpack": dram.tile(
                    [BLK, WB_MAX], bf16, kind="ExternalInput",
                    name="bpack", uniquify=False,
                ),
                "cpack": dram.tile(
                    [BLK, NC_MAX * PW], fp16, kind="ExternalOutput",
                    name="cpack", uniquify=False,
                ),
            }
            pid = nc.partition_id()
            pools = (apool, bpool, cpool, psum_pool)
            tc.switch_hint({e: pid for e in mybir.ALL_ENGINES}, N_CORES, label="core")
            for c in tc.Switch(pid, N_CORES, hint="core"):
                _emit_core(nc, tc, pools, dram_io, c)
    nc.compile()
    return nc


_cached_nc = None

# Optional profiling knobs (used by test.py; harness leaves them off).
TRACE = False
TRACE_KW = {}
LAST_RESULTS = None


def _get_nc():
    global _cached_nc
    if _cached_nc is None:
        _cached_nc = _build()
    return _cached_nc


def _host_pack(A, B):
    import ml_dtypes

    bf16 = ml_dtypes.bfloat16
    AT = np.ascontiguousarray(A.T).astype(bf16)
    Bb = B.astype(bf16)
    apacks, bpacks = [], []
    for c in range(N_CORES):
        ap = np.zeros((BLK, NA_MAX, BLK), dtype=bf16)
        for (q, i), idx in _a_layout(c).items():
            ap[:, idx, :] = AT[q * BLK : (q + 1) * BLK, i * BLK : (i + 1) * BLK]
        bp = np.zeros((BLK, WB_MAX), dtype=bf16)
        boff, _, _ = _b_layout(c)
        for p, act in _phases(c):
            for q, c0, wid in _strips(p, act[-1]):
                w0 = boff[(p, q)]
                bp[:, w0 : w0 + wid] = Bb[q * BLK : (q + 1) * BLK, c0 : c0 + wid]
        apacks.append(ap)
        bpacks.append(bp)
    return apacks, bpacks


def kernel(A, B):
    from concourse.bass_utils import run_bass_kernel_spmd

    A = np.asarray(A, dtype=np.float32)
    B = np.asarray(B, dtype=np.float32)
    nc = _get_nc()
    apacks, bpacks = _host_pack(A, B)
    in_maps = [{"apack": apacks[c], "bpack": bpacks[c]} for c in range(N_CORES)]
    res = run_bass_kernel_spmd(
        nc, in_maps, core_ids=list(range(N_CORES)), trace=TRACE, **TRACE_KW
    )
    global LAST_RESULTS
    LAST_RESULTS = res

    C = np.zeros((N, N), dtype=np.float32)
    for c in range(N_CORES):
        cp = res.results[c]["cpack"]
        for (p, i), j in _c_layout(c).items():
            mr = max(0, i - 4 * p) * BLK
            C[i * BLK : (i + 1) * BLK, p * PW + mr : (p + 1) * PW] = cp[
                :, j * PW + mr : (j + 1) * PW
            ].astype(np.float32)
    return np.triu(C)
